# revision 5
# baseline (speedup 1.0000x reference)
"""ABMIL attention pooling kernel for Trainium2 (8 NeuronCores, data-parallel over slides).

Per core: one slide x[N=16384, F=1024] f32.
  h = gelu(x @ W1 + b1)                    (N, 256)
  aV = tanh(h @ Wv + bv); aU = sigmoid(h @ Wu + bu)
  s = (aV*aU) @ Ww + bw; s += (mask-1)*1e30
  attn = softmax(s); out = attn @ h

On-chip layout: everything transposed (feature dims on partitions, tokens on the
free axis) so the big matmul's contraction (feature dim) sits on PE partitions.
x tiles are transposed on the PE (is_transpose matmuls vs identity, bf16).
"""

import os
import sys

for _p in ("/opt/trn_rl_repo", "/root/.axon_site/_ro/trn_rl_repo"):
    if os.path.isdir(_p) and _p not in sys.path:
        sys.path.insert(0, _p)

import numpy as np

import concourse.bass as bass
import concourse.bass_isa as bass_isa
import concourse.mybir as mybir
from concourse import bacc
from concourse.bass_utils import run_bass_kernel_spmd
from concourse.masks import make_identity
from concourse.tile import TileContext

F32 = mybir.dt.float32
BF16 = mybir.dt.bfloat16
U8 = mybir.dt.uint8
AF = mybir.ActivationFunctionType
ALU = mybir.AluOpType

B = 8
N_TOK = 16384
F = 1024
H = 256
D = 128
KO = F // 128  # 8 feature blocks
HO = H // 128  # 2 hidden blocks
CH = 512       # tokens per chunk
NEG = -1.0e30

_NC_CACHE = {}
LAST_RESULT = None


def build_nc(n_tok=N_TOK, use_gelu=True):
    nch = n_tok // CH        # chunks
    J = n_tok // 128         # 128-token blocks
    nc = bacc.Bacc("TRN2", target_bir_lowering=False, debug=False)

    x_d = nc.dram_tensor("x", [n_tok, F], F32, kind="ExternalInput")
    mask_d = nc.dram_tensor("mask", [n_tok], U8, kind="ExternalInput")
    W1_d = nc.dram_tensor("W1", [F, H], F32, kind="ExternalInput")
    b1_d = nc.dram_tensor("b1", [H], F32, kind="ExternalInput")
    Wv_d = nc.dram_tensor("Wv", [H, D], F32, kind="ExternalInput")
    bv_d = nc.dram_tensor("bv", [D], F32, kind="ExternalInput")
    Wu_d = nc.dram_tensor("Wu", [H, D], F32, kind="ExternalInput")
    bu_d = nc.dram_tensor("bu", [D], F32, kind="ExternalInput")
    Ww_d = nc.dram_tensor("Ww", [D, 1], F32, kind="ExternalInput")
    bw_d = nc.dram_tensor("bw", [1], F32, kind="ExternalInput")
    tok_d = nc.dram_tensor("out_tok", [H], F32, kind="ExternalOutput")
    attn_d = nc.dram_tensor("out_attn", [n_tok], F32, kind="ExternalOutput")

    with TileContext(nc) as tc:
        with (
            tc.tile_pool(name="consts", bufs=1) as cpool,
            tc.tile_pool(name="xa", bufs=3) as xa_pool,
            tc.tile_pool(name="xt", bufs=2) as xt_pool,
            tc.tile_pool(name="act", bufs=3) as act_pool,
            tc.tile_pool(name="psA", bufs=2, space="PSUM") as psA,   # transposes / bcast
            tc.tile_pool(name="psH", bufs=2, space="PSUM") as psH,   # hT accumulators
            tc.tile_pool(name="psV", bufs=2, space="PSUM") as psV,   # aV/aU
            tc.tile_pool(name="psS", bufs=1, space="PSUM") as psS,   # scores + misc
        ):
            # ---------------- constants / weights ----------------
            W1sb = cpool.tile([128, KO, H], BF16)
            nc.gpsimd.dma_start(W1sb, W1_d.ap().rearrange("(ko ki) h -> ki ko h", ki=128))
            Wvsb = cpool.tile([128, HO, D], BF16)
            nc.gpsimd.dma_start(Wvsb, Wv_d.ap().rearrange("(ho hi) d -> hi ho d", hi=128))
            Wusb = cpool.tile([128, HO, D], BF16)
            nc.gpsimd.dma_start(Wusb, Wu_d.ap().rearrange("(ho hi) d -> hi ho d", hi=128))
            Wwsb = cpool.tile([128, 1], BF16)
            nc.gpsimd.dma_start(Wwsb, Ww_d[:, :])

            b1sb = cpool.tile([128, HO], F32)
            for ho in range(HO):
                nc.sync.dma_start(b1sb[:, ho : ho + 1],
                                  b1_d[ho * 128 : (ho + 1) * 128].rearrange("(a b) -> a b", b=1))
            bvsb = cpool.tile([128, 1], F32)
            nc.sync.dma_start(bvsb, bv_d.ap().rearrange("(a b) -> a b", b=1))
            busb = cpool.tile([128, 1], F32)
            nc.sync.dma_start(busb, bu_d.ap().rearrange("(a b) -> a b", b=1))
            bwsb = cpool.tile([1, 1], F32)
            nc.sync.dma_start(bwsb, bw_d.ap().rearrange("(a b) -> a b", b=1))
            bw_b = cpool.tile([128, 1], F32)
            nc.gpsimd.partition_broadcast(bw_b, bwsb)

            ident = cpool.tile([128, 128], BF16)
            make_identity(nc, ident)
            ones_row_bf = cpool.tile([1, 128], BF16)
            nc.gpsimd.memset(ones_row_bf, 1.0)

            # mask -> additive term in score layout [q, j] (tok = j*128 + q)
            maskN = cpool.tile([J, 128], U8)
            nc.sync.dma_start(maskN, mask_d.ap().rearrange("(j q) -> j q", q=128))
            maskNbf = cpool.tile([J, 128], BF16)
            nc.vector.tensor_copy(maskNbf, maskN)
            maskT_ps = psA.tile([128, J], BF16, tag="tp")
            nc.tensor.transpose(maskT_ps, maskNbf, ident[:J, :J])
            maskterm = cpool.tile([128, J], F32)
            nc.scalar.activation(maskterm, maskT_ps, AF.Copy, bias=-1.0e30, scale=1.0e30)

            # ---------------- persistent state ----------------
            hT = cpool.tile([128, HO, n_tok], BF16)       # gelu output, transposed
            S_ps = psS.tile([128, J], F32, tag="s")       # scores [q, j]
            pool_parts = cpool.tile([128, HO, nch], F32)  # per-chunk weighted sums
            pool_acc = cpool.tile([128, HO], F32)         # reduced accumulator

            # ---------------- main streaming loop ----------------
            for c in range(nch):
                xa = xa_pool.tile([128, 4, F], BF16, tag="xa")
                nc.gpsimd.dma_start(
                    xa, x_d[c * CH : (c + 1) * CH, :].rearrange("(s p) f -> p s f", p=128)
                )
                xt = xt_pool.tile([128, KO, CH], BF16, tag="xt")
                for k in range(KO):
                    tp = psA.tile([128, CH], BF16, tag="tp")
                    for s in range(4):
                        nc.tensor.transpose(
                            tp[:, s * 128 : (s + 1) * 128],
                            xa[:, s, k * 128 : (k + 1) * 128],
                            ident,
                        )
                    if k % 2 == 0:
                        nc.vector.tensor_copy(xt[:, k, :], tp)
                    else:
                        nc.scalar.copy(xt[:, k, :], tp)

                for half in range(HO):
                    hps = psH.tile([128, CH], F32, tag="hps")
                    for k in range(KO):
                        nc.tensor.matmul(
                            hps,
                            W1sb[:, k, half * 128 : (half + 1) * 128],
                            xt[:, k, :],
                            start=(k == 0),
                            stop=(k == KO - 1),
                        )
                    nc.scalar.activation(
                        hT[:, half, c * CH : (c + 1) * CH], hps,
                        AF.Gelu if use_gelu else AF.Relu,
                        bias=b1sb[:, half : half + 1],
                    )

                avps = psV.tile([128, CH], F32, tag="avps")
                for half in range(HO):
                    nc.tensor.matmul(
                        avps, Wvsb[:, half, :], hT[:, half, c * CH : (c + 1) * CH],
                        start=(half == 0), stop=(half == HO - 1),
                    )
                av = act_pool.tile([128, CH], BF16, tag="av")
                nc.scalar.activation(av, avps, AF.Tanh, bias=bvsb)

                aups = psV.tile([128, CH], F32, tag="avps")
                for half in range(HO):
                    nc.tensor.matmul(
                        aups, Wusb[:, half, :], hT[:, half, c * CH : (c + 1) * CH],
                        start=(half == 0), stop=(half == HO - 1),
                    )
                au = act_pool.tile([128, CH], BF16, tag="au")
                nc.scalar.activation(au, aups, AF.Sigmoid, bias=busb)

                gt = act_pool.tile([128, CH], BF16, tag="gt")
                nc.vector.tensor_mul(gt, av, au)

                for s in range(4):
                    nc.tensor.matmul(
                        S_ps[:, c * 4 + s : c * 4 + s + 1],
                        gt[:, s * 128 : (s + 1) * 128],
                        Wwsb,
                        start=True, stop=True,
                    )

            # ---------------- softmax tail ----------------
            S2 = cpool.tile([128, J], F32)
            nc.vector.tensor_add(S2, S_ps, maskterm)
            Wbf = cpool.tile([128, J], BF16)     # exp(scores), [q, j]
            Zp = cpool.tile([128, 1], F32)
            nc.scalar.activation(Wbf, S2, AF.Exp, bias=bw_b, accum_out=Zp)

            Zall = cpool.tile([128, 1], F32)
            nc.gpsimd.partition_all_reduce(Zall, Zp, channels=128,
                                           reduce_op=bass_isa.ReduceOp.add)
            invZ = cpool.tile([128, 1], F32)
            nc.vector.reciprocal(invZ, Zall)

            WT_ps = psA.tile([J, 128], BF16, tag="tp")
            nc.tensor.transpose(WT_ps, Wbf, ident)
            WTsb = cpool.tile([J, 128], BF16)    # [j, q] = token order
            nc.vector.tensor_copy(WTsb, WT_ps)

            attnT = cpool.tile([J, 128], F32)
            nc.scalar.activation(attnT, WTsb, AF.Copy, bias=0.0, scale=invZ[:J, :])
            nc.sync.dma_start(attn_d.ap().rearrange("(j q) -> j q", q=128), attnT)

            wflat = cpool.tile([1, n_tok], BF16)
            nc.sync.dma_start(wflat, WTsb)

            # ---------------- weighted pooling ----------------
            for c in range(nch):
                bcb = act_pool.tile([128, CH], BF16, tag="bcb")
                nc.gpsimd.partition_broadcast(bcb, wflat[:, c * CH : (c + 1) * CH])
                for half in range(HO):
                    ttro = act_pool.tile([128, CH], BF16, tag="ttro")
                    nc.vector.scalar_tensor_tensor(
                        out=ttro,
                        in0=hT[:, half, c * CH : (c + 1) * CH],
                        scalar=1.0,
                        in1=bcb,
                        op0=ALU.mult,
                        op1=ALU.mult,
                        accum_out=pool_parts[:, half, c : c + 1],
                    )
            for half in range(HO):
                nc.vector.tensor_reduce(
                    pool_acc[:, half : half + 1], pool_parts[:, half, :],
                    axis=mybir.AxisListType.X, op=ALU.add,
                )

            pooled = cpool.tile([128, HO], F32)
            nc.scalar.activation(pooled, pool_acc, AF.Copy, bias=0.0, scale=invZ)
            for ho in range(HO):
                nc.sync.dma_start(
                    tok_d[ho * 128 : (ho + 1) * 128].rearrange("(a b) -> a b", b=1),
                    pooled[:, ho : ho + 1],
                )

    nc.compile()
    return nc


def _get_nc(n_tok=N_TOK):
    if n_tok not in _NC_CACHE:
        _NC_CACHE[n_tok] = build_nc(n_tok)
    return _NC_CACHE[n_tok]


def kernel(x, mask, W1, b1, Wv, bv, Wu, bu, Ww, bw):
    global LAST_RESULT
    x = np.ascontiguousarray(np.asarray(x, dtype=np.float32))
    mask_u8 = np.ascontiguousarray(np.asarray(mask).astype(np.uint8))
    W1 = np.ascontiguousarray(np.asarray(W1, dtype=np.float32))
    b1 = np.ascontiguousarray(np.asarray(b1, dtype=np.float32))
    Wv = np.ascontiguousarray(np.asarray(Wv, dtype=np.float32))
    bv = np.ascontiguousarray(np.asarray(bv, dtype=np.float32))
    Wu = np.ascontiguousarray(np.asarray(Wu, dtype=np.float32))
    bu = np.ascontiguousarray(np.asarray(bu, dtype=np.float32))
    Ww = np.ascontiguousarray(np.asarray(Ww, dtype=np.float32))
    bw = np.ascontiguousarray(np.asarray(bw, dtype=np.float32))

    n_tok = x.shape[1]
    nc = _get_nc(n_tok)
    in_maps = []
    for b in range(B):
        in_maps.append({
            "x": x[b], "mask": mask_u8[b],
            "W1": W1, "b1": b1, "Wv": Wv, "bv": bv,
            "Wu": Wu, "bu": bu, "Ww": Ww, "bw": bw,
        })
    res = run_bass_kernel_spmd(nc, in_maps, core_ids=list(range(B)))
    LAST_RESULT = res
    toks = np.stack([r["out_tok"] for r in res.results])[:, None, :]
    attn = np.stack([r["out_attn"] for r in res.results])[:, None, :]
    return toks.astype(np.float32), attn.astype(np.float32)
